# revision 1
# baseline (speedup 1.0000x reference)
"""Single-head attention (B=8, N=2048, D=1024) on 8 TRN2 NeuronCores.

Strategy: pure data-parallel over batch (B=8 == n_cores). Each core runs one
batch element end-to-end; no collectives.

Per-core math (b = core index):
    qkv = x[b] @ W_qkv.T + b_qkv          # [N, 3D]
    q, k, v = split(qkv)                   # each [N, D]
    S = q @ k.T / sqrt(D)                  # [N, N]
    P = exp(S)   (no max-subtraction: |S| <~ 6 for randn inputs, safe in f32)
    out[b] = (P @ v) / rowsum(P)

Device layouts (chosen so every matmul contracts over the partition dim):
    xt  = x[b].T           [D, N]   (c on partitions)   bf16
    wt  = W_qkv.T          [D, 3D]  (c on partitions)   bf16
    QT/KT (on SBUF)        [d, N]   (d on partitions)   bf16
    V (on SBUF)            [N, D]   (keys on partitions) bf16
    S^T blocks             [keys 128, queries 512]  (exp is elementwise; the
        rowsum over keys is done with a ones-weights matmul that also
        broadcasts the sum across all 128 partitions)
    outT                   [D, N]  f32, host transposes back

All matmuls are bf16 with fp32 PSUM accumulation; inputs are cast to bf16 on
the host (host-side shard prep), output returned in fp32.
"""

import numpy as np
import ml_dtypes

import concourse.bass as bass
import concourse.mybir as mybir
import concourse.tile as tile
from concourse import bacc
from concourse.bass_utils import run_bass_kernel_spmd

P = 128
N = 2048          # sequence length per core
D = 1024          # head dim
O = 3 * D         # qkv projection output dim
CT = D // P       # 8 contraction tiles for the projection
F = 512           # matmul moving free dim (one fp32 PSUM bank)
NT = N // F       # 4 n-tiles in phase 1 / q-tiles in phase 2
KTILES = N // P   # 16 key tiles of 128
DT = D // P       # 8 d tiles of 128
SCALE = 1.0 / float(D) ** 0.5

BF16 = mybir.dt.bfloat16
F32 = mybir.dt.float32
NP_BF16 = ml_dtypes.bfloat16

# Cache of (nc, ) so repeated kernel() calls don't recompile.
_COMPILED = None
LAST_RESULT = None  # test harness reads exec_time_ns off this


def _build():
    nc = bacc.Bacc("TRN2", target_bir_lowering=False, debug=False, num_devices=8)

    xt_d = nc.declare_dram_parameter("xt", [D, N], BF16, isOutput=False)
    wt_d = nc.declare_dram_parameter("wt", [D, O], BF16, isOutput=False)
    bqk_d = nc.declare_dram_parameter("bqk", [P, 2 * DT], F32, isOutput=False)
    bv_d = nc.declare_dram_parameter("bv", [P, D], F32, isOutput=False)
    out_d = nc.declare_dram_parameter("outt", [D, N], F32, isOutput=True)

    xt_r = xt_d.ap().rearrange("(co p) n -> p co n", p=P)     # [128, 8, N]
    wt_r = wt_d.ap().rearrange("(co p) o -> p co o", p=P)     # [128, 8, O]
    out_r = out_d.ap().rearrange("(dc p) n -> p dc n", p=P)   # [128, 8, N]

    IDENT = mybir.ActivationFunctionType.Identity
    EXP = mybir.ActivationFunctionType.Exp

    with tile.TileContext(nc) as tc:
        with tc.tile_pool(name="persist", bufs=1) as persist:
            bqk = persist.tile([P, 2 * DT], F32)
            nc.sync.dma_start(bqk[:, :], bqk_d.ap()[:, :])
            bv = persist.tile([P, D], F32)
            nc.sync.dma_start(bv[:, :], bv_d.ap()[:, :])
            ones = persist.tile([P, P], BF16)
            nc.vector.memset(ones[:, :], 1.0)

            QT = persist.tile([P, DT, N], BF16)
            KT = persist.tile([P, DT, N], BF16)
            V = persist.tile([P, KTILES, D], BF16)

            # ---------------- phase 1: qkv projection ----------------
            with (
                tc.tile_pool(name="phase1", bufs=1) as p1,
                tc.tile_pool(name="psum1", bufs=3, space="PSUM") as psum1,
            ):
                x_t = []
                w_t = []
                for c in range(CT):
                    xt = p1.tile([P, N], BF16, tag=f"x{c}")
                    nc.sync.dma_start(xt[:, :], xt_r[:, c, :])
                    x_t.append(xt)
                    wt = p1.tile([P, O], BF16, tag=f"w{c}")
                    nc.sync.dma_start(wt[:, :], wt_r[:, c, :])
                    w_t.append(wt)

                for nt in range(NT):
                    nsl = slice(nt * F, (nt + 1) * F)
                    # Q^T and K^T: out [o 128, n 512]
                    for ot in range(2 * DT):
                        ps = psum1.tile([P, F], F32, tag="ps")
                        for c in range(CT):
                            nc.tensor.matmul(
                                ps[:, :],
                                lhsT=w_t[c][:, ot * P:(ot + 1) * P],
                                rhs=x_t[c][:, nsl],
                                start=(c == 0),
                                stop=(c == CT - 1),
                            )
                        dest = QT if ot < DT else KT
                        col = ot % DT
                        nc.scalar.activation(
                            dest[:, col, nsl], ps[:, :], IDENT,
                            bias=bqk[:, ot:ot + 1], scale=1.0,
                        )
                    # V: out [n 128, d 512]
                    for u in range(F // P):
                        ng = nt * (F // P) + u
                        nb = nt * F + u * P
                        for dh in range(D // F):
                            dsl = slice(dh * F, (dh + 1) * F)
                            ps = psum1.tile([P, F], F32, tag="psv")
                            for c in range(CT):
                                nc.tensor.matmul(
                                    ps[:, :],
                                    lhsT=x_t[c][:, nb:nb + P],
                                    rhs=w_t[c][:, 2 * D + dh * F:2 * D + (dh + 1) * F],
                                    start=(c == 0),
                                    stop=(c == CT - 1),
                                )
                            nc.vector.tensor_add(V[:, ng, dsl], ps[:, :], bv[:, dsl])

            # ---------------- phase 2: attention ----------------
            with (
                tc.tile_pool(name="phase2", bufs=2) as p2,
                tc.tile_pool(name="psum2", bufs=3, space="PSUM") as psum2,
                tc.tile_pool(name="psumr", bufs=2, space="PSUM") as psumr,
            ):
                for qt in range(NT):
                    qsl = slice(qt * F, (qt + 1) * F)
                    ps_r = psumr.tile([P, F], F32, tag="ps_r")
                    pt_tiles = []
                    for kt in range(KTILES):
                        ps_s = psum2.tile([P, F], F32, tag="ps_s")
                        for dt in range(DT):
                            nc.tensor.matmul(
                                ps_s[:, :],
                                lhsT=KT[:, dt, kt * P:(kt + 1) * P],
                                rhs=QT[:, dt, qsl],
                                start=(dt == 0),
                                stop=(dt == DT - 1),
                            )
                        pt = p2.tile([P, F], BF16, tag=f"pt{kt}")
                        nc.scalar.activation(pt[:, :], ps_s[:, :], EXP, scale=SCALE)
                        # rowsum over keys, broadcast to all 128 partitions
                        nc.tensor.matmul(
                            ps_r[:, :], lhsT=ones[:, :], rhs=pt[:, :],
                            start=(kt == 0), stop=(kt == KTILES - 1),
                        )
                        pt_tiles.append(pt)
                    recip = p2.tile([P, F], F32, tag="recip")
                    nc.vector.reciprocal(recip[:, :], ps_r[:, :])
                    for dc in range(DT):
                        ps_o = psum2.tile([P, F], F32, tag="ps_o")
                        for kt in range(KTILES):
                            nc.tensor.matmul(
                                ps_o[:, :],
                                lhsT=V[:, kt, dc * P:(dc + 1) * P],
                                rhs=pt_tiles[kt][:, :],
                                start=(kt == 0),
                                stop=(kt == KTILES - 1),
                            )
                        ob = p2.tile([P, F], F32, tag="ob")
                        nc.vector.tensor_mul(ob[:, :], ps_o[:, :], recip[:, :])
                        nc.sync.dma_start(out_r[:, dc, qsl], ob[:, :])

    nc.compile()
    return nc


def _get_compiled():
    global _COMPILED
    if _COMPILED is None:
        _COMPILED = _build()
    return _COMPILED


def kernel(x, W_qkv, b_qkv, trace=False):
    global LAST_RESULT
    x = np.asarray(x, dtype=np.float32)
    W_qkv = np.asarray(W_qkv, dtype=np.float32)
    b_qkv = np.asarray(b_qkv, dtype=np.float32)
    B = x.shape[0]
    assert x.shape == (8, N, D) and W_qkv.shape == (O, D) and b_qkv.shape == (O,)

    nc = _get_compiled()

    wt = np.ascontiguousarray(W_qkv.T).astype(NP_BF16)            # [D, O]
    bqk = np.ascontiguousarray(
        b_qkv[:2 * D].reshape(2 * DT, P).T.astype(np.float32))    # [128, 16]
    bv = np.ascontiguousarray(
        np.broadcast_to(b_qkv[2 * D:].astype(np.float32), (P, D)))  # [128, D]

    in_maps = []
    for b in range(B):
        xt = np.ascontiguousarray(x[b].T).astype(NP_BF16)         # [D, N]
        in_maps.append({"xt": xt, "wt": wt, "bqk": bqk, "bv": bv})

    res = run_bass_kernel_spmd(nc, in_maps, core_ids=list(range(8)), trace=trace)
    LAST_RESULT = res

    out = np.stack([res.results[b]["outt"].T for b in range(B)])  # [8, N, D]
    return np.ascontiguousarray(out.astype(np.float32))


# revision 5
# speedup vs baseline: 1.0230x; 1.0230x over previous
"""Single-head attention (B=8, N=2048, D=1024) on 8 TRN2 NeuronCores.

Strategy: pure data-parallel over batch (B=8 == n_cores). Each core runs one
batch element end-to-end; no collectives.

Per-core math (b = core index):
    qkv = x[b] @ W_qkv.T + b_qkv          # [N, 3D]
    q, k, v = split(qkv)                   # each [N, D]
    S = q @ k.T / sqrt(D)                  # [N, N]
    P = exp(S)   (no max-subtraction: |S| <~ 6 for randn inputs, safe in f32)
    out[b] = (P @ v) / rowsum(P)

Device layouts (chosen so every matmul contracts over the partition dim):
    xt  = x[b].T           [D, N]   (c on partitions)   bf16
    wt  = W_qkv.T          [D, 3D]  (c on partitions)   bf16
    QT/KT (on SBUF)        [d, N]   (d on partitions)   bf16
    V (on SBUF)            [N, D]   (keys on partitions) bf16
    S^T blocks             [keys 128, queries 512]  (exp is elementwise; the
        rowsum over keys is done with a ones-weights matmul that also
        broadcasts the sum across all 128 partitions)
    outT                   [D, N]  f32, host transposes back

All matmuls are bf16 with fp32 PSUM accumulation; inputs are cast to bf16 on
the host (host-side shard prep), output returned in fp32.
"""

import numpy as np
import ml_dtypes

import concourse.bass as bass
import concourse.mybir as mybir
import concourse.tile as tile
from concourse import bacc
from concourse.bass_utils import run_bass_kernel_spmd

P = 128
N = 2048          # sequence length per core
D = 1024          # head dim
O = 3 * D         # qkv projection output dim
CT = D // P       # 8 contraction tiles for the projection
F = 512           # matmul moving free dim (one fp32 PSUM bank)
NT = N // F       # 4 n-tiles in phase 1 / q-tiles in phase 2
KTILES = N // P   # 16 key tiles of 128
DT = D // P       # 8 d tiles of 128
SCALE = 1.0 / float(D) ** 0.5

BF16 = mybir.dt.bfloat16
F32 = mybir.dt.float32
NP_BF16 = ml_dtypes.bfloat16

# Cache of (nc, ) so repeated kernel() calls don't recompile.
_COMPILED = None
LAST_RESULT = None  # test harness reads exec_time_ns off this


def _build():
    nc = bacc.Bacc("TRN2", target_bir_lowering=False, debug=False, num_devices=8)

    xt_d = nc.declare_dram_parameter("xt", [D, N], BF16, isOutput=False)
    wt_d = nc.declare_dram_parameter("wt", [D, O], BF16, isOutput=False)
    bqk_d = nc.declare_dram_parameter("bqk", [P, 2 * DT], F32, isOutput=False)
    bv_d = nc.declare_dram_parameter("bv", [P, D], F32, isOutput=False)
    out_d = nc.declare_dram_parameter("outt", [D, N], F32, isOutput=True)

    xt_r = xt_d.ap().rearrange("(co p) n -> p co n", p=P)     # [128, 8, N]
    wt_r = wt_d.ap().rearrange("(co p) o -> p co o", p=P)     # [128, 8, O]
    out_r = out_d.ap().rearrange("(dc p) n -> p dc n", p=P)   # [128, 8, N]

    IDENT = mybir.ActivationFunctionType.Identity
    EXP = mybir.ActivationFunctionType.Exp

    with tile.TileContext(nc) as tc:
        with tc.tile_pool(name="persist", bufs=1) as persist:
            bqk = persist.tile([P, 2 * DT], F32)
            nc.sync.dma_start(bqk[:, :], bqk_d.ap()[:, :])
            bv = persist.tile([P, D], F32)
            nc.sync.dma_start(bv[:, :], bv_d.ap()[:, :])
            ones32 = persist.tile([P, P], F32)
            nc.vector.memset(ones32[:, :], 1.0)

            QT = persist.tile([P, DT, N], BF16)
            KT = persist.tile([P, DT, N], BF16)
            V = persist.tile([P, KTILES, D], BF16)

            # ---------------- phase 1: qkv projection ----------------
            with (
                tc.tile_pool(name="phase1", bufs=1) as p1,
                tc.tile_pool(name="psum1", bufs=3, space="PSUM") as psum1,
            ):
                # x triggers on the sync queue engine, W on gpsimd: DMA
                # triggers cost ~0.7-1.8us each on their sequencer, so
                # serializing all 16 on one engine delays the first matmul.
                x_t = []
                w_t = []
                for c in range(CT):
                    xt = p1.tile([P, N], BF16, tag=f"x{c}")
                    nc.sync.dma_start(xt[:, :], xt_r[:, c, :])
                    x_t.append(xt)
                    wt = p1.tile([P, O], BF16, tag=f"w{c}")
                    nc.gpsimd.dma_start(wt[:, :], wt_r[:, c, :])
                    w_t.append(wt)

                for nt in range(NT):
                    nsl = slice(nt * F, (nt + 1) * F)
                    # Q^T and K^T: out [o 128, n 512]
                    for ot in range(2 * DT):
                        ps = psum1.tile([P, F], F32, tag="ps")
                        for c in range(CT):
                            nc.tensor.matmul(
                                ps[:, :],
                                lhsT=w_t[c][:, ot * P:(ot + 1) * P],
                                rhs=x_t[c][:, nsl],
                                start=(c == 0),
                                stop=(c == CT - 1),
                            )
                        dest = QT if ot < DT else KT
                        col = ot % DT
                        nc.scalar.activation(
                            dest[:, col, nsl], ps[:, :], IDENT,
                            bias=bqk[:, ot:ot + 1], scale=1.0,
                        )
                    # V: out [n 128, d 512]
                    for u in range(F // P):
                        ng = nt * (F // P) + u
                        nb = nt * F + u * P
                        for dh in range(D // F):
                            dsl = slice(dh * F, (dh + 1) * F)
                            ps = psum1.tile([P, F], F32, tag="psv")
                            for c in range(CT):
                                nc.tensor.matmul(
                                    ps[:, :],
                                    lhsT=x_t[c][:, nb:nb + P],
                                    rhs=w_t[c][:, 2 * D + dh * F:2 * D + (dh + 1) * F],
                                    start=(c == 0),
                                    stop=(c == CT - 1),
                                )
                            nc.vector.tensor_add(V[:, ng, dsl], ps[:, :], bv[:, dsl])

            # ---------------- phase 2: attention ----------------
            with (
                tc.tile_pool(name="phase2", bufs=2) as p2,
                tc.tile_pool(name="psum2", bufs=3, space="PSUM") as psum2,
                tc.tile_pool(name="psumr", bufs=2, space="PSUM") as psumr,
            ):
                for qt in range(NT):
                    qsl = slice(qt * F, (qt + 1) * F)
                    acc = p2.tile([P, F], F32, tag="acc")
                    pt_tiles = []
                    for kt in range(KTILES):
                        ps_s = psum2.tile([P, F], F32, tag="ps_s")
                        for dt in range(DT):
                            nc.tensor.matmul(
                                ps_s[:, :],
                                lhsT=KT[:, dt, kt * P:(kt + 1) * P],
                                rhs=QT[:, dt, qsl],
                                start=(dt == 0),
                                stop=(dt == DT - 1),
                            )
                        pt = p2.tile([P, F], BF16, tag=f"pt{kt}")
                        nc.scalar.activation(pt[:, :], ps_s[:, :], EXP, scale=SCALE)
                        # per-partition partial rowsums on DVE (cheap, idle
                        # engine) so the partition-reduce below is one matmul
                        # instead of 16
                        if kt == 0:
                            nc.vector.tensor_copy(acc[:, :], pt[:, :])
                        else:
                            nc.vector.tensor_add(acc[:, :], acc[:, :], pt[:, :])
                        pt_tiles.append(pt)
                    # reduce over partitions + broadcast to all 128: ones.T @ acc
                    ps_r = psumr.tile([P, F], F32, tag="ps_r")
                    nc.tensor.matmul(ps_r[:, :], lhsT=ones32[:, :], rhs=acc[:, :],
                                     start=True, stop=True)
                    recip = p2.tile([P, F], F32, tag="recip")
                    nc.vector.reciprocal(recip[:, :], ps_r[:, :])
                    for dc in range(DT):
                        ps_o = psum2.tile([P, F], F32, tag="ps_o")
                        for kt in range(KTILES):
                            nc.tensor.matmul(
                                ps_o[:, :],
                                lhsT=V[:, kt, dc * P:(dc + 1) * P],
                                rhs=pt_tiles[kt][:, :],
                                start=(kt == 0),
                                stop=(kt == KTILES - 1),
                            )
                        ob = p2.tile([P, F], F32, tag="ob")
                        nc.vector.tensor_mul(ob[:, :], ps_o[:, :], recip[:, :])
                        # alternate trigger engines so output DMAs don't queue
                        # behind each other on one sequencer at the tail
                        eng = nc.sync if (qt * DT + dc) % 2 == 0 else nc.gpsimd
                        eng.dma_start(out_r[:, dc, qsl], ob[:, :])

    nc.compile()
    return nc


def _get_compiled():
    global _COMPILED
    if _COMPILED is None:
        _COMPILED = _build()
    return _COMPILED


def kernel(x, W_qkv, b_qkv, trace=False):
    global LAST_RESULT
    x = np.asarray(x, dtype=np.float32)
    W_qkv = np.asarray(W_qkv, dtype=np.float32)
    b_qkv = np.asarray(b_qkv, dtype=np.float32)
    B = x.shape[0]
    assert x.shape == (8, N, D) and W_qkv.shape == (O, D) and b_qkv.shape == (O,)

    nc = _get_compiled()

    wt = np.ascontiguousarray(W_qkv.T).astype(NP_BF16)            # [D, O]
    bqk = np.ascontiguousarray(
        b_qkv[:2 * D].reshape(2 * DT, P).T.astype(np.float32))    # [128, 16]
    bv = np.ascontiguousarray(
        np.broadcast_to(b_qkv[2 * D:].astype(np.float32), (P, D)))  # [128, D]

    in_maps = []
    for b in range(B):
        xt = np.ascontiguousarray(x[b].T).astype(NP_BF16)         # [D, N]
        in_maps.append({"xt": xt, "wt": wt, "bqk": bqk, "bv": bv})

    res = run_bass_kernel_spmd(nc, in_maps, core_ids=list(range(8)), trace=trace)
    LAST_RESULT = res

    out = np.stack([res.results[b]["outt"].T for b in range(B)])  # [8, N, D]
    return np.ascontiguousarray(out.astype(np.float32))


# revision 8
# speedup vs baseline: 1.0339x; 1.0107x over previous
"""Single-head attention (B=8, N=2048, D=1024) on 8 TRN2 NeuronCores.

Strategy: pure data-parallel over batch (B=8 == n_cores). Each core runs one
batch element end-to-end; no collectives.

Per-core math (b = core index):
    qkv = x[b] @ W_qkv.T + b_qkv          # [N, 3D]
    q, k, v = split(qkv)                   # each [N, D]
    S = q @ k.T / sqrt(D)                  # [N, N]
    P = exp(S)   (no max-subtraction: |S| <~ 6 for randn inputs, safe in f32)
    out[b] = (P @ v) / rowsum(P)

Device layouts (chosen so every matmul contracts over the partition dim):
    xt  = x[b].T           [D, N]   (c on partitions)   bf16
    wt  = W_qkv.T          [D, 3D]  (c on partitions)   bf16
    QT/KT (on SBUF)        [d, N]   (d on partitions)   bf16
    V (on SBUF)            [N, D]   (keys on partitions) bf16
    S^T blocks             [keys 128, queries 512]  (exp is elementwise; the
        rowsum over keys is done with a ones-weights matmul that also
        broadcasts the sum across all 128 partitions)
    outT                   [D, N]  f32, host transposes back

All matmuls are bf16 with fp32 PSUM accumulation; inputs are cast to bf16 on
the host (host-side shard prep), output returned in fp32.
"""

import numpy as np
import ml_dtypes

import concourse.bass as bass
import concourse.mybir as mybir
import concourse.tile as tile
from concourse import bacc
from concourse.bass_utils import run_bass_kernel_spmd

P = 128
N = 2048          # sequence length per core
D = 1024          # head dim
O = 3 * D         # qkv projection output dim
CT = D // P       # 8 contraction tiles for the projection
F = 512           # matmul moving free dim (one fp32 PSUM bank)
NT = N // F       # 4 n-tiles in phase 1 / q-tiles in phase 2
KTILES = N // P   # 16 key tiles of 128
DT = D // P       # 8 d tiles of 128
SCALE = 1.0 / float(D) ** 0.5

BF16 = mybir.dt.bfloat16
F32 = mybir.dt.float32
NP_BF16 = ml_dtypes.bfloat16

# Cache of (nc, ) so repeated kernel() calls don't recompile.
_COMPILED = None
LAST_RESULT = None  # test harness reads exec_time_ns off this


def _build():
    nc = bacc.Bacc("TRN2", target_bir_lowering=False, debug=False, num_devices=8)

    xt_d = nc.declare_dram_parameter("xt", [D, N], BF16, isOutput=False)
    wt_d = nc.declare_dram_parameter("wt", [D, O], BF16, isOutput=False)
    bqk_d = nc.declare_dram_parameter("bqk", [P, 2 * DT], F32, isOutput=False)
    bv_d = nc.declare_dram_parameter("bv", [P, D], F32, isOutput=False)
    out_d = nc.declare_dram_parameter("outt", [D, N], F32, isOutput=True)

    xt_r = xt_d.ap().rearrange("(co p) n -> p co n", p=P)     # [128, 8, N]
    wt_r = wt_d.ap().rearrange("(co p) o -> p co o", p=P)     # [128, 8, O]
    out_r = out_d.ap().rearrange("(dc p) n -> p dc n", p=P)   # [128, 8, N]

    IDENT = mybir.ActivationFunctionType.Identity
    EXP = mybir.ActivationFunctionType.Exp

    with tile.TileContext(nc) as tc:
        with tc.tile_pool(name="persist", bufs=1) as persist:
            bqk = persist.tile([P, 2 * DT], F32)
            nc.gpsimd.dma_start(bqk[:, :], bqk_d.ap()[:, :])
            bv = persist.tile([P, D], F32)
            nc.gpsimd.dma_start(bv[:, :], bv_d.ap()[:, :])
            ones32 = persist.tile([P, P], F32)
            nc.vector.memset(ones32[:, :], 1.0)

            QT = persist.tile([P, DT, N], BF16)
            KT = persist.tile([P, DT, N], BF16)
            V = persist.tile([P, KTILES, D], BF16)

            # ---------------- phase 1: qkv projection ----------------
            with (
                tc.tile_pool(name="phase1", bufs=1) as p1,
                tc.tile_pool(name="psum1", bufs=3, space="PSUM") as psum1,
            ):
                # split input triggers across both HWDGE engines (sync +
                # scalar): a dma_start trigger costs ~0.7us serial on its
                # engine, so 16 on one engine delays the first matmul.
                x_t = []
                w_t = []
                for c in range(CT):
                    xt = p1.tile([P, N], BF16, tag=f"x{c}")
                    nc.sync.dma_start(xt[:, :], xt_r[:, c, :])
                    x_t.append(xt)
                    wt = p1.tile([P, O], BF16, tag=f"w{c}")
                    nc.scalar.dma_start(wt[:, :], wt_r[:, c, :])
                    w_t.append(wt)

                for nt in range(NT):
                    nsl = slice(nt * F, (nt + 1) * F)
                    # Q^T and K^T: out [o 128, n 512]
                    for ot in range(2 * DT):
                        ps = psum1.tile([P, F], F32, tag="ps")
                        for c in range(CT):
                            nc.tensor.matmul(
                                ps[:, :],
                                lhsT=w_t[c][:, ot * P:(ot + 1) * P],
                                rhs=x_t[c][:, nsl],
                                start=(c == 0),
                                stop=(c == CT - 1),
                            )
                        dest = QT if ot < DT else KT
                        col = ot % DT
                        nc.scalar.activation(
                            dest[:, col, nsl], ps[:, :], IDENT,
                            bias=bqk[:, ot:ot + 1], scale=1.0,
                        )
                    # V: out [n 128, d 512]
                    for u in range(F // P):
                        ng = nt * (F // P) + u
                        nb = nt * F + u * P
                        for dh in range(D // F):
                            dsl = slice(dh * F, (dh + 1) * F)
                            ps = psum1.tile([P, F], F32, tag="psv")
                            for c in range(CT):
                                nc.tensor.matmul(
                                    ps[:, :],
                                    lhsT=x_t[c][:, nb:nb + P],
                                    rhs=w_t[c][:, 2 * D + dh * F:2 * D + (dh + 1) * F],
                                    start=(c == 0),
                                    stop=(c == CT - 1),
                                )
                            nc.vector.tensor_add(V[:, ng, dsl], ps[:, :], bv[:, dsl])

            # ---------------- phase 2: attention ----------------
            with (
                tc.tile_pool(name="phase2", bufs=2) as p2,
                tc.tile_pool(name="psum2", bufs=3, space="PSUM") as psum2,
                tc.tile_pool(name="psumr", bufs=2, space="PSUM") as psumr,
            ):
                for qt in range(NT):
                    qsl = slice(qt * F, (qt + 1) * F)
                    acc = p2.tile([P, F], F32, tag="acc")
                    pt_tiles = []
                    for kt in range(KTILES):
                        ps_s = psum2.tile([P, F], F32, tag="ps_s")
                        for dt in range(DT):
                            nc.tensor.matmul(
                                ps_s[:, :],
                                lhsT=KT[:, dt, kt * P:(kt + 1) * P],
                                rhs=QT[:, dt, qsl],
                                start=(dt == 0),
                                stop=(dt == DT - 1),
                            )
                        pt = p2.tile([P, F], BF16, tag=f"pt{kt}")
                        nc.scalar.activation(pt[:, :], ps_s[:, :], EXP, scale=SCALE)
                        # per-partition partial rowsums on DVE (cheap, idle
                        # engine) so the partition-reduce below is one matmul
                        # instead of 16
                        if kt == 0:
                            nc.vector.tensor_copy(acc[:, :], pt[:, :])
                        else:
                            nc.vector.tensor_add(acc[:, :], acc[:, :], pt[:, :])
                        pt_tiles.append(pt)
                    # reduce over partitions + broadcast to all 128: ones.T @ acc
                    ps_r = psumr.tile([P, F], F32, tag="ps_r")
                    nc.tensor.matmul(ps_r[:, :], lhsT=ones32[:, :], rhs=acc[:, :],
                                     start=True, stop=True)
                    recip = p2.tile([P, F], F32, tag="recip")
                    nc.vector.reciprocal(recip[:, :], ps_r[:, :])
                    for dc in range(DT):
                        ps_o = psum2.tile([P, F], F32, tag="ps_o")
                        for kt in range(KTILES):
                            nc.tensor.matmul(
                                ps_o[:, :],
                                lhsT=V[:, kt, dc * P:(dc + 1) * P],
                                rhs=pt_tiles[kt][:, :],
                                start=(kt == 0),
                                stop=(kt == KTILES - 1),
                            )
                        ob = p2.tile([P, F], F32, tag="ob")
                        nc.vector.tensor_mul(ob[:, :], ps_o[:, :], recip[:, :])
                        nc.sync.dma_start(out_r[:, dc, qsl], ob[:, :])

    nc.compile()
    return nc


def _get_compiled():
    global _COMPILED
    if _COMPILED is None:
        _COMPILED = _build()
    return _COMPILED


def kernel(x, W_qkv, b_qkv, trace=False):
    global LAST_RESULT
    x = np.asarray(x, dtype=np.float32)
    W_qkv = np.asarray(W_qkv, dtype=np.float32)
    b_qkv = np.asarray(b_qkv, dtype=np.float32)
    B = x.shape[0]
    assert x.shape == (8, N, D) and W_qkv.shape == (O, D) and b_qkv.shape == (O,)

    nc = _get_compiled()

    wt = np.ascontiguousarray(W_qkv.T).astype(NP_BF16)            # [D, O]
    bqk = np.ascontiguousarray(
        b_qkv[:2 * D].reshape(2 * DT, P).T.astype(np.float32))    # [128, 16]
    bv = np.ascontiguousarray(
        np.broadcast_to(b_qkv[2 * D:].astype(np.float32), (P, D)))  # [128, D]

    in_maps = []
    for b in range(B):
        xt = np.ascontiguousarray(x[b].T).astype(NP_BF16)         # [D, N]
        in_maps.append({"xt": xt, "wt": wt, "bqk": bqk, "bv": bv})

    res = run_bass_kernel_spmd(nc, in_maps, core_ids=list(range(8)), trace=trace)
    LAST_RESULT = res

    out = np.stack([res.results[b]["outt"].T for b in range(B)])  # [8, N, D]
    return np.ascontiguousarray(out.astype(np.float32))


# revision 10
# speedup vs baseline: 1.2206x; 1.1806x over previous
"""Single-head attention (B=8, N=2048, D=1024) on 8 TRN2 NeuronCores.

Strategy: pure data-parallel over batch (B=8 == n_cores). Each core runs one
batch element end-to-end; no collectives.

Per-core math (b = core index):
    qkv = x[b] @ W_qkv.T + b_qkv          # [N, 3D]
    q, k, v = split(qkv)                   # each [N, D]
    S = q @ k.T / sqrt(D)                  # [N, N]
    P = exp(S)   (no max-subtraction: |S| <~ 6 for randn inputs, safe in f32)
    out[b] = (P @ v) / rowsum(P)

Device layouts (chosen so every matmul contracts over the partition dim):
    xt  = x[b].T           [D, N]   (c on partitions)   bf16
    wt  = W_qkv.T          [D, 3D]  (c on partitions)   bf16
    QT/KT (on SBUF)        [d, N]   (d on partitions)   bf16
    V (on SBUF)            [N, D]   (keys on partitions) bf16
    S^T blocks             [keys 128, queries 512]  (exp is elementwise; the
        rowsum over keys is done with a ones-weights matmul that also
        broadcasts the sum across all 128 partitions)
    outT                   [D, N]  f32, host transposes back

All matmuls are bf16 with fp32 PSUM accumulation; inputs are cast to bf16 on
the host (host-side shard prep), output returned in fp32.
"""

import numpy as np
import ml_dtypes

import concourse.bass as bass
import concourse.mybir as mybir
import concourse.tile as tile
from concourse import bacc
from concourse.bass_utils import run_bass_kernel_spmd

P = 128
N = 2048          # sequence length per core
D = 1024          # head dim
O = 3 * D         # qkv projection output dim
CT = D // P       # 8 contraction tiles for the projection
F = 512           # matmul moving free dim (one fp32 PSUM bank)
NT = N // F       # 4 n-tiles in phase 1 / q-tiles in phase 2
KTILES = N // P   # 16 key tiles of 128
DT = D // P       # 8 d tiles of 128
SCALE = 1.0 / float(D) ** 0.5

BF16 = mybir.dt.bfloat16
F32 = mybir.dt.float32
NP_BF16 = ml_dtypes.bfloat16

# Cache of (nc, ) so repeated kernel() calls don't recompile.
_COMPILED = None
LAST_RESULT = None  # test harness reads exec_time_ns off this


def _build():
    nc = bacc.Bacc("TRN2", target_bir_lowering=False, debug=False, num_devices=8)

    xt_d = nc.declare_dram_parameter("xt", [D, N], BF16, isOutput=False)
    wt_d = nc.declare_dram_parameter("wt", [D, O], BF16, isOutput=False)
    bqk_d = nc.declare_dram_parameter("bqk", [P, 2 * DT], F32, isOutput=False)
    bv_d = nc.declare_dram_parameter("bv", [P, D], F32, isOutput=False)
    out_d = nc.declare_dram_parameter("outt", [D, N], F32, isOutput=True)

    xt_r = xt_d.ap().rearrange("(co p) n -> p co n", p=P)     # [128, 8, N]
    wt_r = wt_d.ap().rearrange("(co p) o -> p co o", p=P)     # [128, 8, O]
    out_r = out_d.ap().rearrange("(dc p) n -> p dc n", p=P)   # [128, 8, N]

    IDENT = mybir.ActivationFunctionType.Identity
    EXP = mybir.ActivationFunctionType.Exp

    with tile.TileContext(nc) as tc:
        with tc.tile_pool(name="persist", bufs=1) as persist:
            bqk = persist.tile([P, 2 * DT], F32)
            nc.gpsimd.dma_start(bqk[:, :], bqk_d.ap()[:, :])
            bv = persist.tile([P, D], F32)
            nc.gpsimd.dma_start(bv[:, :], bv_d.ap()[:, :])
            ones32 = persist.tile([P, P], F32)
            nc.vector.memset(ones32[:, :], 1.0)

            QT = persist.tile([P, DT, N], BF16)
            KT = persist.tile([P, DT, N], BF16)
            V = persist.tile([P, KTILES, D], BF16)

            # ---------------- phase 1: qkv projection ----------------
            with (
                tc.tile_pool(name="phase1", bufs=1) as p1,
                tc.tile_pool(name="psum1", bufs=3, space="PSUM") as psum1,
            ):
                # Input loads, ordered by when phase 1 consumes each range.
                # Triggers are split across both HWDGE engines (sync+scalar;
                # ~0.7us serial per trigger) and each chunk is split into a
                # "first slice" wave (all that's needed to start computing)
                # and a bulk wave. Tile's range-granular deps let the first
                # matmul group start as soon as the first slices land.
                x_t = [p1.tile([P, N], BF16, tag=f"x{c}", name=f"x{c}")
                       for c in range(CT)]
                w_t = [p1.tile([P, O], BF16, tag=f"w{c}", name=f"w{c}")
                       for c in range(CT)]
                for c in range(CT):
                    nc.sync.dma_start(x_t[c][:, 0:F], xt_r[:, c, 0:F])
                    nc.scalar.dma_start(w_t[c][:, 0:2 * F], wt_r[:, c, 0:2 * F])
                for c in range(CT):
                    nc.sync.dma_start(x_t[c][:, F:N], xt_r[:, c, F:N])
                    nc.scalar.dma_start(w_t[c][:, 2 * F:O], wt_r[:, c, 2 * F:O])

                for nt in range(NT):
                    nsl = slice(nt * F, (nt + 1) * F)
                    # Q^T and K^T: out [o 128, n 512]
                    for ot in range(2 * DT):
                        ps = psum1.tile([P, F], F32, tag="ps")
                        for c in range(CT):
                            nc.tensor.matmul(
                                ps[:, :],
                                lhsT=w_t[c][:, ot * P:(ot + 1) * P],
                                rhs=x_t[c][:, nsl],
                                start=(c == 0),
                                stop=(c == CT - 1),
                            )
                        dest = QT if ot < DT else KT
                        col = ot % DT
                        nc.scalar.activation(
                            dest[:, col, nsl], ps[:, :], IDENT,
                            bias=bqk[:, ot:ot + 1], scale=1.0,
                        )
                    # V: out [n 128, d 512]
                    for u in range(F // P):
                        ng = nt * (F // P) + u
                        nb = nt * F + u * P
                        for dh in range(D // F):
                            dsl = slice(dh * F, (dh + 1) * F)
                            ps = psum1.tile([P, F], F32, tag="psv")
                            for c in range(CT):
                                nc.tensor.matmul(
                                    ps[:, :],
                                    lhsT=x_t[c][:, nb:nb + P],
                                    rhs=w_t[c][:, 2 * D + dh * F:2 * D + (dh + 1) * F],
                                    start=(c == 0),
                                    stop=(c == CT - 1),
                                )
                            nc.vector.tensor_add(V[:, ng, dsl], ps[:, :], bv[:, dsl])

            # ---------------- phase 2: attention ----------------
            with (
                tc.tile_pool(name="phase2", bufs=2) as p2,
                tc.tile_pool(name="psum2", bufs=3, space="PSUM") as psum2,
                tc.tile_pool(name="psumr", bufs=2, space="PSUM") as psumr,
            ):
                for qt in range(NT):
                    qsl = slice(qt * F, (qt + 1) * F)
                    acc = p2.tile([P, F], F32, tag="acc")
                    pt_tiles = []
                    for kt in range(KTILES):
                        ps_s = psum2.tile([P, F], F32, tag="ps_s")
                        for dt in range(DT):
                            nc.tensor.matmul(
                                ps_s[:, :],
                                lhsT=KT[:, dt, kt * P:(kt + 1) * P],
                                rhs=QT[:, dt, qsl],
                                start=(dt == 0),
                                stop=(dt == DT - 1),
                            )
                        pt = p2.tile([P, F], BF16, tag=f"pt{kt}")
                        nc.scalar.activation(pt[:, :], ps_s[:, :], EXP, scale=SCALE)
                        # per-partition partial rowsums on DVE (cheap, idle
                        # engine) so the partition-reduce below is one matmul
                        # instead of 16
                        if kt == 0:
                            nc.vector.tensor_copy(acc[:, :], pt[:, :])
                        else:
                            nc.vector.tensor_add(acc[:, :], acc[:, :], pt[:, :])
                        pt_tiles.append(pt)
                    # reduce over partitions + broadcast to all 128: ones.T @ acc
                    ps_r = psumr.tile([P, F], F32, tag="ps_r")
                    nc.tensor.matmul(ps_r[:, :], lhsT=ones32[:, :], rhs=acc[:, :],
                                     start=True, stop=True)
                    recip = p2.tile([P, F], F32, tag="recip")
                    nc.vector.reciprocal(recip[:, :], ps_r[:, :])
                    for dc in range(DT):
                        ps_o = psum2.tile([P, F], F32, tag="ps_o")
                        for kt in range(KTILES):
                            nc.tensor.matmul(
                                ps_o[:, :],
                                lhsT=V[:, kt, dc * P:(dc + 1) * P],
                                rhs=pt_tiles[kt][:, :],
                                start=(kt == 0),
                                stop=(kt == KTILES - 1),
                            )
                        ob = p2.tile([P, F], F32, tag="ob")
                        nc.vector.tensor_mul(ob[:, :], ps_o[:, :], recip[:, :])
                        nc.sync.dma_start(out_r[:, dc, qsl], ob[:, :])

    nc.compile()
    return nc


def _get_compiled():
    global _COMPILED
    if _COMPILED is None:
        _COMPILED = _build()
    return _COMPILED


def kernel(x, W_qkv, b_qkv, trace=False):
    global LAST_RESULT
    x = np.asarray(x, dtype=np.float32)
    W_qkv = np.asarray(W_qkv, dtype=np.float32)
    b_qkv = np.asarray(b_qkv, dtype=np.float32)
    B = x.shape[0]
    assert x.shape == (8, N, D) and W_qkv.shape == (O, D) and b_qkv.shape == (O,)

    nc = _get_compiled()

    wt = np.ascontiguousarray(W_qkv.T).astype(NP_BF16)            # [D, O]
    bqk = np.ascontiguousarray(
        b_qkv[:2 * D].reshape(2 * DT, P).T.astype(np.float32))    # [128, 16]
    bv = np.ascontiguousarray(
        np.broadcast_to(b_qkv[2 * D:].astype(np.float32), (P, D)))  # [128, D]

    in_maps = []
    for b in range(B):
        xt = np.ascontiguousarray(x[b].T).astype(NP_BF16)         # [D, N]
        in_maps.append({"xt": xt, "wt": wt, "bqk": bqk, "bv": bv})

    res = run_bass_kernel_spmd(nc, in_maps, core_ids=list(range(8)), trace=trace)
    LAST_RESULT = res

    out = np.stack([res.results[b]["outt"].T for b in range(B)])  # [8, N, D]
    return np.ascontiguousarray(out.astype(np.float32))


# revision 11
# speedup vs baseline: 1.2421x; 1.0176x over previous
"""Single-head attention (B=8, N=2048, D=1024) on 8 TRN2 NeuronCores.

Strategy: pure data-parallel over batch (B=8 == n_cores). Each core runs one
batch element end-to-end; no collectives.

Per-core math (b = core index):
    qkv = x[b] @ W_qkv.T + b_qkv          # [N, 3D]
    q, k, v = split(qkv)                   # each [N, D]
    S = q @ k.T / sqrt(D)                  # [N, N]
    P = exp(S)   (no max-subtraction: |S| <~ 6 for randn inputs, safe in f32)
    out[b] = (P @ v) / rowsum(P)

Device layouts (chosen so every matmul contracts over the partition dim):
    xt  = x[b].T           [D, N]   (c on partitions)   bf16
    wt  = W_qkv.T          [D, 3D]  (c on partitions)   bf16
    QT/KT (on SBUF)        [d, N]   (d on partitions)   bf16
    V (on SBUF)            [N, D]   (keys on partitions) bf16
    S^T blocks             [keys 128, queries 512]  (exp is elementwise; the
        rowsum over keys is done with a ones-weights matmul that also
        broadcasts the sum across all 128 partitions)
    outT                   [D, N]  f32, host transposes back

All matmuls are bf16 with fp32 PSUM accumulation; inputs are cast to bf16 on
the host (host-side shard prep), output returned in fp32.
"""

import numpy as np
import ml_dtypes

import concourse.bass as bass
import concourse.mybir as mybir
import concourse.tile as tile
from concourse import bacc
from concourse.bass_utils import run_bass_kernel_spmd

P = 128
N = 2048          # sequence length per core
D = 1024          # head dim
O = 3 * D         # qkv projection output dim
CT = D // P       # 8 contraction tiles for the projection
F = 512           # matmul moving free dim (one fp32 PSUM bank)
NT = N // F       # 4 n-tiles in phase 1 / q-tiles in phase 2
KTILES = N // P   # 16 key tiles of 128
DT = D // P       # 8 d tiles of 128
SCALE = 1.0 / float(D) ** 0.5

BF16 = mybir.dt.bfloat16
F32 = mybir.dt.float32
NP_BF16 = ml_dtypes.bfloat16

# Cache of (nc, ) so repeated kernel() calls don't recompile.
_COMPILED = None
LAST_RESULT = None  # test harness reads exec_time_ns off this


def _build():
    nc = bacc.Bacc("TRN2", target_bir_lowering=False, debug=False, num_devices=8)

    xt_d = nc.declare_dram_parameter("xt", [D, N], BF16, isOutput=False)
    wt_d = nc.declare_dram_parameter("wt", [D, O], BF16, isOutput=False)
    bqk_d = nc.declare_dram_parameter("bqk", [P, 2 * DT], F32, isOutput=False)
    bv_d = nc.declare_dram_parameter("bv", [P, D], F32, isOutput=False)
    out_d = nc.declare_dram_parameter("outt", [D, N], F32, isOutput=True)

    xt_r = xt_d.ap().rearrange("(co p) n -> p co n", p=P)     # [128, 8, N]
    wt_r = wt_d.ap().rearrange("(co p) o -> p co o", p=P)     # [128, 8, O]
    out_r = out_d.ap().rearrange("(dc p) n -> p dc n", p=P)   # [128, 8, N]

    IDENT = mybir.ActivationFunctionType.Identity
    EXP = mybir.ActivationFunctionType.Exp

    with tile.TileContext(nc) as tc:
        with tc.tile_pool(name="persist", bufs=1) as persist:
            bqk = persist.tile([P, 2 * DT], F32)
            nc.gpsimd.dma_start(bqk[:, :], bqk_d.ap()[:, :])
            bv = persist.tile([P, D], F32)
            nc.gpsimd.dma_start(bv[:, :], bv_d.ap()[:, :])
            ones32 = persist.tile([P, P], F32)
            nc.vector.memset(ones32[:, :], 1.0)

            QT = persist.tile([P, DT, N], BF16)
            KT = persist.tile([P, DT, N], BF16)
            V = persist.tile([P, KTILES, D], BF16)

            # ---------------- phase 1: qkv projection ----------------
            with (
                tc.tile_pool(name="phase1", bufs=1) as p1,
                tc.tile_pool(name="psum1", bufs=3, space="PSUM") as psum1,
            ):
                # Input loads, ordered by when phase 1 consumes each range.
                # Triggers are split across both HWDGE engines (sync+scalar;
                # ~0.7us serial per trigger) and each chunk is split into a
                # "first slice" wave (all that's needed to start computing)
                # and a bulk wave. Tile's range-granular deps let the first
                # matmul group start as soon as the first slices land.
                x_t = [p1.tile([P, N], BF16, tag=f"x{c}", name=f"x{c}")
                       for c in range(CT)]
                w_t = [p1.tile([P, O], BF16, tag=f"w{c}", name=f"w{c}")
                       for c in range(CT)]
                # wave order matches phase-1 consumption: x[0:512] + w[0:1024]
                # feed the first ~25us of matmuls, then w[1024:2048], then the
                # V weights w[2048:3072], and x[512:2048] last (needed only
                # from the second n-tile onward)
                for c in range(CT):
                    nc.sync.dma_start(x_t[c][:, 0:F], xt_r[:, c, 0:F])
                    nc.scalar.dma_start(w_t[c][:, 0:2 * F], wt_r[:, c, 0:2 * F])
                for c in range(CT):
                    nc.sync.dma_start(w_t[c][:, 2 * F:4 * F], wt_r[:, c, 2 * F:4 * F])
                    nc.scalar.dma_start(w_t[c][:, 4 * F:O], wt_r[:, c, 4 * F:O])
                for c in range(CT):
                    nc.sync.dma_start(x_t[c][:, F:N], xt_r[:, c, F:N])

                for nt in range(NT):
                    nsl = slice(nt * F, (nt + 1) * F)
                    # Q^T and K^T: out [o 128, n 512]
                    for ot in range(2 * DT):
                        ps = psum1.tile([P, F], F32, tag="ps")
                        for c in range(CT):
                            nc.tensor.matmul(
                                ps[:, :],
                                lhsT=w_t[c][:, ot * P:(ot + 1) * P],
                                rhs=x_t[c][:, nsl],
                                start=(c == 0),
                                stop=(c == CT - 1),
                            )
                        dest = QT if ot < DT else KT
                        col = ot % DT
                        nc.scalar.activation(
                            dest[:, col, nsl], ps[:, :], IDENT,
                            bias=bqk[:, ot:ot + 1], scale=1.0,
                        )
                    # V: out [n 128, d 512]
                    for u in range(F // P):
                        ng = nt * (F // P) + u
                        nb = nt * F + u * P
                        for dh in range(D // F):
                            dsl = slice(dh * F, (dh + 1) * F)
                            ps = psum1.tile([P, F], F32, tag="psv")
                            for c in range(CT):
                                nc.tensor.matmul(
                                    ps[:, :],
                                    lhsT=x_t[c][:, nb:nb + P],
                                    rhs=w_t[c][:, 2 * D + dh * F:2 * D + (dh + 1) * F],
                                    start=(c == 0),
                                    stop=(c == CT - 1),
                                )
                            nc.vector.tensor_add(V[:, ng, dsl], ps[:, :], bv[:, dsl])

            # ---------------- phase 2: attention ----------------
            with (
                tc.tile_pool(name="phase2", bufs=2) as p2,
                tc.tile_pool(name="psum2", bufs=3, space="PSUM") as psum2,
                tc.tile_pool(name="psumr", bufs=2, space="PSUM") as psumr,
            ):
                for qt in range(NT):
                    qsl = slice(qt * F, (qt + 1) * F)
                    acc = p2.tile([P, F], F32, tag="acc")
                    pt_tiles = []
                    for kt in range(KTILES):
                        ps_s = psum2.tile([P, F], F32, tag="ps_s")
                        for dt in range(DT):
                            nc.tensor.matmul(
                                ps_s[:, :],
                                lhsT=KT[:, dt, kt * P:(kt + 1) * P],
                                rhs=QT[:, dt, qsl],
                                start=(dt == 0),
                                stop=(dt == DT - 1),
                            )
                        pt = p2.tile([P, F], BF16, tag=f"pt{kt}")
                        nc.scalar.activation(pt[:, :], ps_s[:, :], EXP, scale=SCALE)
                        # per-partition partial rowsums on DVE (cheap, idle
                        # engine) so the partition-reduce below is one matmul
                        # instead of 16
                        if kt == 0:
                            nc.vector.tensor_copy(acc[:, :], pt[:, :])
                        else:
                            nc.vector.tensor_add(acc[:, :], acc[:, :], pt[:, :])
                        pt_tiles.append(pt)
                    # reduce over partitions + broadcast to all 128: ones.T @ acc
                    ps_r = psumr.tile([P, F], F32, tag="ps_r")
                    nc.tensor.matmul(ps_r[:, :], lhsT=ones32[:, :], rhs=acc[:, :],
                                     start=True, stop=True)
                    recip = p2.tile([P, F], F32, tag="recip")
                    nc.vector.reciprocal(recip[:, :], ps_r[:, :])
                    for dc in range(DT):
                        ps_o = psum2.tile([P, F], F32, tag="ps_o")
                        for kt in range(KTILES):
                            nc.tensor.matmul(
                                ps_o[:, :],
                                lhsT=V[:, kt, dc * P:(dc + 1) * P],
                                rhs=pt_tiles[kt][:, :],
                                start=(kt == 0),
                                stop=(kt == KTILES - 1),
                            )
                        ob = p2.tile([P, F], F32, tag="ob")
                        nc.vector.tensor_mul(ob[:, :], ps_o[:, :], recip[:, :])
                        nc.sync.dma_start(out_r[:, dc, qsl], ob[:, :])

    nc.compile()
    return nc


def _get_compiled():
    global _COMPILED
    if _COMPILED is None:
        _COMPILED = _build()
    return _COMPILED


def kernel(x, W_qkv, b_qkv, trace=False):
    global LAST_RESULT
    x = np.asarray(x, dtype=np.float32)
    W_qkv = np.asarray(W_qkv, dtype=np.float32)
    b_qkv = np.asarray(b_qkv, dtype=np.float32)
    B = x.shape[0]
    assert x.shape == (8, N, D) and W_qkv.shape == (O, D) and b_qkv.shape == (O,)

    nc = _get_compiled()

    wt = np.ascontiguousarray(W_qkv.T).astype(NP_BF16)            # [D, O]
    bqk = np.ascontiguousarray(
        b_qkv[:2 * D].reshape(2 * DT, P).T.astype(np.float32))    # [128, 16]
    bv = np.ascontiguousarray(
        np.broadcast_to(b_qkv[2 * D:].astype(np.float32), (P, D)))  # [128, D]

    in_maps = []
    for b in range(B):
        xt = np.ascontiguousarray(x[b].T).astype(NP_BF16)         # [D, N]
        in_maps.append({"xt": xt, "wt": wt, "bqk": bqk, "bv": bv})

    res = run_bass_kernel_spmd(nc, in_maps, core_ids=list(range(8)), trace=trace)
    LAST_RESULT = res

    out = np.stack([res.results[b]["outt"].T for b in range(B)])  # [8, N, D]
    return np.ascontiguousarray(out.astype(np.float32))
